# revision 17
# baseline (speedup 1.0000x reference)
"""Trainium2 Bass kernel for a CGNS block (GNN message passing).

Math: the reference builds A = a a^T + I (rank-1 + identity), L = D^-1/2 A D^-1/2,
then out = relu(BN(conv1x1(cat[x@A, (L@x^T)^T]))).  Exploiting the rank-1
structure, with a = relu(tanh(w)), S = sum(a), d_n = 1/sqrt(a_n*S + 1),
u = d*a, s0 = x@a, s1 = x@u, the whole block collapses to

  y[:, n] = W1~ x[:, n] + d2[n] * (W2~ x[:, n]) + a[n] v1 + u[n] v2 + b~
  out     = relu(y)

where W~ are the BN-folded conv weights, v1 = W1~ s0, v2 = W2~ s1.  No [N,N]
matrix is ever materialized.

Sharding: 8 cores; core i handles batch b = i//2, half h = i%2 of the N=4096
node dim (2048 columns each).  Each core reads the full x[b] once in
transposed bf16 layout (feeds the s0/s1 reduction) and its own half in
natural bf16 layout (for the main matmuls).  n-chunks are rolled per-core so
chunks 0..15 are always the core's own half -> identical SPMD program.

v5 changes over the v4 baseline (32.2us -> target ~22us):
 - all-bf16 datapath: xh/wvb/xa/wAB/out in bf16.  Halves the dominant DMA
   traffic (input 1.11MB -> 0.82MB, output 512KB -> 256KB) and removes the
   fp32r 4x/row PE penalty (moving free dim 128 < 256 threshold).
 - w arrives pre-transposed from host as wcol [128, 32] fp32 (contiguous,
   128B rows): kills the PE transpose + PSUM hop that gated tanh ~0.9us.
 - inputs split across both HW queues with the critical tensors first:
   sync = wcol, xt[0:16], wvb, xh[:1024]; scalar = wrow, xt[16:32],
   xh[1024:] (xt lands ~10.4 vs ~13.9 in v4).
 - d2 = reciprocal_approx_fast (custom DVE, no table, ~5x faster than the
   table reciprocal); d = sqrt(d2) on scalar as before.
 - ONE v-matmul (s01 [64,66] stationary with col64=s0, col65=s1) instead of
   two, and v2 lands in wAB[65, 0:64] (the y1 half).  v4 had v2 in the q
   half, which wrongly scaled the u*v2 term by d2 (benign at ~4e-4 rel, but
   free accuracy here).
 - relus on the scalar engine; STT epilogue writes bf16.
"""

import numpy as np

import concourse.bacc as bacc
import concourse.bass as bass
import concourse.tile as tile
from concourse import mybir

FP = mybir.dt.float32
BF = mybir.dt.bfloat16
B, C, N = 4, 64, 4096
NH = N // 2          # columns per core
JH = NH // 128       # 16 chunks per core half
JF = N // 128        # 32 chunks full N
BN_EPS = 1e-5


def build_nc():
    nc = bacc.Bacc()
    AF = mybir.ActivationFunctionType
    OP = mybir.AluOpType
    AX = mybir.AxisListType

    # DRAM I/O (per-core shards supplied via in_maps)
    xt = nc.dram_tensor("xt", [128, JF, C], BF, kind="ExternalInput")
    # xh rows 0:64 = x half; row 66 = ones (host-baked); rows 64:65 zeros
    # (overwritten on device by the a/u row scatters)
    xh = nc.dram_tensor("xh", [67, NH], BF, kind="ExternalInput")
    wcol = nc.dram_tensor("wcol", [128, JF], FP, kind="ExternalInput")
    wrow = nc.dram_tensor("wrow", [JH, 128], FP, kind="ExternalInput")
    wvb = nc.dram_tensor("wvb", [67, 2 * C], BF, kind="ExternalInput")
    out = nc.dram_tensor("out", [128, JH, C], BF, kind="ExternalOutput")

    with tile.TileContext(nc) as tc:
        with (
            tc.tile_pool(name="sb", bufs=1) as sb,
            tc.tile_pool(name="ps", bufs=1, space="PSUM") as ps,
        ):
            # SBUF tiles
            xt_sb = sb.tile([128, JF, C], BF, name="xt_sb")
            # xa: rows 0:64 = x half (natural), 64 = a, 65 = u, 66 = 1
            xa = sb.tile([67, NH], BF, name="xa")
            wcol_sb = sb.tile([128, JF], FP, name="wcol_sb")
            wrow_sb = sb.tile([JH, 128], FP, name="wrow_sb")
            # wAB rows 0:64 = [W1~T | W2~T] (host), 64 = [v1|0], 65 = [v2|0]
            # (device; host preloads zeros), 66 = [b~ | 0] (host)
            wAB = sb.tile([67, 2 * C], BF, name="wAB")
            tcol = sb.tile([128, JF], FP, name="tcol")
            acol = sb.tile([128, JF], FP, name="acol")
            d2col = sb.tile([128, JF], FP, name="d2col")
            dcol = sb.tile([128, JF], FP, name="dcol")
            au_bf = sb.tile([128, 2 * JF], BF, name="au_bf")
            trow = sb.tile([JH, 128], FP, name="trow")
            arow_bf = sb.tile([JH, 128], BF, name="arow_bf")
            d2row = sb.tile([JH, 128], FP, name="d2row")
            drow = sb.tile([JH, 128], FP, name="drow")
            urow_bf = sb.tile([JH, 128], BF, name="urow_bf")
            ones_bf = sb.tile([128, 128], BF, name="ones_bf")
            apart = sb.tile([128, 1], FP, name="apart")
            apart_r = sb.tile([128, 2], BF, name="apart_r")
            sS = sb.tile([128, 1], FP, name="sS")
            # v-matmul stationaries: 66 cols of zeros except col 64 = s0 (A)
            # resp. col 65 = s1 (B) -> v1/v2 at PSUM partitions 64/65.  The
            # two v-matmuls accumulate into the same [66, 64] PSUM region so
            # row 64 = v1, row 65 = v2 both land in cols 0:64 (one aligned
            # 2-partition copy; engines can't start writes at partition 65).
            s0A = sb.tile([C, 66], BF, name="s0A")
            s1B = sb.tile([C, 66], BF, name="s1B")
            t1 = sb.tile([128, JH * C], FP, name="t1")
            yp = sb.tile([128, JH * C], BF, name="yp")
            yo = sb.tile([128, JH * C], BF, name="yo")

            # PSUM tiles (each padded to a bank; 7 banks total)
            p_sm = ps.tile([128, 2], FP, name="p_sm")
            p_s = ps.tile([C, 2], FP, name="p_s")
            p_v = ps.tile([66, C], FP, name="p_v")
            p_yq = [ps.tile([128, 512], FP, name=f"p_yq_{g}") for g in range(4)]

            # ---- DMA issues -------------------------------------------------
            # sync HW queue: wcol first (gates the whole scalar chain), then
            # its half of xt (gates the s-reduction), wvb, its half of xh.
            nc.sync.dma_start(wcol_sb[:], wcol[:])
            nc.sync.dma_start(xt_sb[:, 0:16, :], xt[:, 0:16, :])
            nc.sync.dma_start(wAB[:], wvb[:])
            nc.sync.dma_start(xa[:, 0:1024], xh[:, 0:1024])
            # scalar HW queue: wrow first (gates the row chain), then the
            # other xt half, the other xh half.  tanh runs between the xt and
            # xh descriptor writes so it isn't delayed past wcol's landing.
            nc.scalar.dma_start(wrow_sb[:], wrow[:])
            nc.scalar.dma_start(xt_sb[:, 16:32, :], xt[:, 16:32, :])

            # ---- tanh (scalar; wcol/wrow land first on their queues) -------
            nc.scalar.activation(tcol[:], wcol_sb[:], AF.Tanh)
            nc.scalar.activation(trow[:], wrow_sb[:], AF.Tanh)
            nc.scalar.dma_start(xa[:, 1024:2048], xh[:, 1024:2048])

            # ---- constants (vector prologue) -------------------------------
            nc.vector.memset(ones_bf[:], 1.0)
            nc.vector.memset(s0A[:], 0.0)
            nc.vector.memset(s1B[:], 0.0)

            # ---- column chain: a, S, d2, d, u ------------------------------
            nc.vector.tensor_scalar_max(acol[:], tcol[:], 0.0)
            au_v = au_bf[:].rearrange("p (c s) -> p c s", s=2)
            nc.vector.tensor_copy(au_v[:, :, 0], acol[:])
            nc.vector.tensor_reduce(apart[:], acol[:], axis=AX.X, op=OP.add)
            nc.vector.tensor_copy(apart_r[:], apart[:].broadcast_to([128, 2]))
            # S broadcast to all partitions (tiny matmul; high priority so the
            # list scheduler doesn't sink it behind the s accumulation group)
            with tc.high_priority():
                nc.tensor.matmul(
                    p_sm[:], ones_bf[:], apart_r[:], start=True, stop=True
                )
            # sS copy only feeds the gpsimd row path; the column chain
            # reads S straight from PSUM
            nc.vector.tensor_copy(sS[:], p_sm[:, 0:1])
            # row-path relu here fills the vector gap while S broadcasts
            # (bf16 out so the scatter needs no cast and can use a HW queue
            # -- SWDGE completion semaphores cost ~1.3us before consumers)
            nc.vector.tensor_scalar_max(arow_bf[:], trow[:], 0.0)
            # t = a*S + 1 ; d2 = 1/t ; d = sqrt(d2) ; u = d*a
            nc.vector.tensor_scalar(
                tcol[:], acol[:], p_sm[:, 0:1], 1.0, op0=OP.mult, op1=OP.add
            )
            nc.vector.reciprocal_approx_fast(d2col[:], tcol[:])
            nc.scalar.sqrt(dcol[:], d2col[:])
            nc.vector.tensor_mul(au_v[:, :, 1], dcol[:], acol[:])

            # ---- row chain (a/u rows of xa).  t-row on gpsimd so the
            # in-order vector stream doesn't stall; sqrt on scalar; both
            # scatters ride the HW queues (bf16, no cast needed).
            nc.sync.dma_start(xa[64:65, :], arow_bf[:])
            nc.gpsimd.tensor_scalar(
                trow[:], arow_bf[:], sS[0:JH, :], 1.0, op0=OP.mult, op1=OP.add
            )
            nc.vector.reciprocal_approx_fast(d2row[:], trow[:])
            nc.scalar.sqrt(drow[:], d2row[:])
            nc.vector.tensor_mul(urow_bf[:], drow[:], arow_bf[:])
            nc.scalar.dma_start(xa[65:66, :], urow_bf[:])

            # ---- s0/s1 reduction over full N (PE, accumulate in PSUM) ------
            for j in range(JF):
                nc.tensor.matmul(
                    p_s[:],
                    xt_sb[:, j, :],
                    au_bf[:, 2 * j : 2 * j + 2],
                    start=(j == 0),
                    stop=(j == JF - 1),
                )
            # v-matmuls accumulate into one region: row 64 = v1, row 65 = v2,
            # both in cols 0:64 (v2 against the W2~ block -> correct, unlike
            # v4 where v2 sat in the q half and got d2-scaled).  The copies
            # carry late ready-time hints so the scheduler doesn't slot them
            # early in the in-order vector stream.
            with tc.tile_wait_until(0.0115):
                nc.scalar.copy(s0A[:, 64:65], p_s[:, 0:1])
                nc.vector.tensor_copy(s1B[:, 65:66], p_s[:, 1:2])
            nc.tensor.matmul(
                p_v[:], s0A[:], wAB[0:C, 0:C], start=True, stop=False
            )
            nc.tensor.matmul(
                p_v[:], s1B[:], wAB[0:C, C : 2 * C], start=False, stop=True
            )
            with tc.tile_wait_until(0.0125):
                nc.vector.tensor_copy(wAB[64:66, 0:C], p_v[64:66, 0:C])

            # ---- main matmuls: one [67,128]x[67,128] mm per chunk.
            # out columns 0:64 = y1 (conv1 + rank-1 + bias), 64:128 = q (conv2)
            for j in range(JH):
                grp, jj = divmod(j, 4)
                nc.tensor.matmul(
                    p_yq[grp][:, 128 * jj : 128 * (jj + 1)],
                    xa[:, 128 * j : 128 * (j + 1)],
                    wAB[:],
                    start=True, stop=True,
                )

            # ---- epilogue: yo = relu(q * d2 + y1) --------------------------
            # Two wide vector ops per 4-chunk group instead of 16 per-chunk
            # STTs + 4 scalar y1-copies: t1 = q * d2 (d2 free-broadcast over
            # the 64 out channels), yp = t1 + y1 with y1 read straight from
            # PSUM (only one PSUM operand per op).
            for g in range(4):
                pv = p_yq[g][:].rearrange("p (j c) -> p j c", c=2 * C)
                t1v = t1[:, 256 * g : 256 * (g + 1)].rearrange(
                    "p (j c) -> p j c", c=C
                )
                ypv = yp[:, 256 * g : 256 * (g + 1)].rearrange(
                    "p (j c) -> p j c", c=C
                )
                d2v = (
                    d2col[:, 4 * g : 4 * (g + 1)]
                    .unsqueeze(2)
                    .broadcast_to([128, 4, C])
                )
                nc.vector.tensor_tensor(
                    t1v, pv[:, :, C : 2 * C], d2v, op=OP.mult
                )
                nc.vector.tensor_tensor(
                    ypv, t1v, pv[:, :, 0:C], op=OP.add
                )
            for g in range(4):
                nc.scalar.activation(
                    yo[:, 256 * g : 256 * (g + 1)],
                    yp[:, 256 * g : 256 * (g + 1)],
                    AF.Relu,
                )
                if g < 3:
                    eng = nc.scalar if g % 2 == 0 else nc.sync
                    eng.dma_start(
                        out[:, 4 * g : 4 * (g + 1), :],
                        yo[:, 256 * g : 256 * (g + 1)].rearrange(
                            "p (j c) -> p j c", c=C
                        ),
                    )
                else:
                    # the last group's transfer is exec-critical: split by
                    # partitions across both HW queues
                    nc.sync.dma_start(
                        out[0:64, 12:16, :],
                        yo[0:64, 768:1024].rearrange("p (j c) -> p j c", c=C),
                    )
                    nc.scalar.dma_start(
                        out[64:128, 12:16, :],
                        yo[64:128, 768:1024].rearrange("p (j c) -> p j c", c=C),
                    )
    nc.compile()
    return nc


def make_in_maps(x, w, conv_w, conv_b, bn_gamma, bn_beta, bn_mean, bn_var):
    import ml_dtypes

    BF_NP = ml_dtypes.bfloat16
    x = np.asarray(x, np.float32)
    w = np.asarray(w, np.float32)
    conv_w = np.asarray(conv_w, np.float32)
    conv_b = np.asarray(conv_b, np.float32)
    bn_gamma = np.asarray(bn_gamma, np.float32)
    bn_beta = np.asarray(bn_beta, np.float32)
    bn_mean = np.asarray(bn_mean, np.float32)
    bn_var = np.asarray(bn_var, np.float32)

    scale = bn_gamma / np.sqrt(bn_var + BN_EPS)
    wmat = conv_w * scale[:, None]                       # [64, 128] BN-folded
    w1t = np.ascontiguousarray(wmat[:, :C].T)            # [c, o]
    w2t = np.ascontiguousarray(wmat[:, C:].T)
    wvb = np.zeros((67, 2 * C), np.float32)
    wvb[0:C] = np.concatenate([w1t, w2t], axis=1)
    wvb[66, :C] = conv_b * scale + bn_beta - bn_mean * scale
    wvb = wvb.astype(BF_NP)

    in_maps = []
    for i in range(8):
        b, h = divmod(i, 2)
        xb = x[b, :, :, 0]                               # [64, 4096]
        order = np.roll(np.arange(JF), -JH * h)          # own half first
        xt_jpc = np.ascontiguousarray(xb.T).reshape(JF, 128, C)
        xt_pjc = np.ascontiguousarray(
            xt_jpc[order].transpose(1, 0, 2).astype(BF_NP)
        )
        xhb = np.zeros((67, NH), np.float32)
        xhb[0:C] = xb[:, NH * h : NH * (h + 1)]
        xhb[66] = 1.0
        xhb = np.ascontiguousarray(xhb.astype(BF_NP))
        wcol = np.ascontiguousarray(w[b].reshape(JF, 128)[order].T)
        wrow = np.ascontiguousarray(w[b][NH * h : NH * (h + 1)].reshape(JH, 128))
        in_maps.append(
            {
                "xt": xt_pjc,
                "xh": xhb,
                "wcol": wcol,
                "wrow": wrow,
                "wvb": wvb,
            }
        )
    return in_maps


def assemble_out(results):
    out = np.empty((B, C, N), np.float32)
    for i in range(8):
        b, h = divmod(i, 2)
        blk = np.asarray(results[i]["out"]).astype(np.float32)  # [128, 16, 64]
        y_half = blk.transpose(1, 0, 2).reshape(NH, C)   # row = 128*j + p
        out[b, :, NH * h : NH * (h + 1)] = y_half.T
    return out[..., None]


_NC = None


def kernel(**inputs):
    global _NC
    from concourse.bass_utils import run_bass_kernel_spmd

    if _NC is None:
        _NC = build_nc()
    in_maps = make_in_maps(**inputs)
    res = run_bass_kernel_spmd(_NC, in_maps, list(range(8)))
    return assemble_out(res.results)


# revision 19
# speedup vs baseline: 1.0003x; 1.0003x over previous
"""Trainium2 Bass kernel for a CGNS block (GNN message passing).

Math: the reference builds A = a a^T + I (rank-1 + identity), L = D^-1/2 A D^-1/2,
then out = relu(BN(conv1x1(cat[x@A, (L@x^T)^T]))).  Exploiting the rank-1
structure, with a = relu(tanh(w)), S = sum(a), d_n = 1/sqrt(a_n*S + 1),
u = d*a, s0 = x@a, s1 = x@u, the whole block collapses to

  y[:, n] = W1~ x[:, n] + d2[n] * (W2~ x[:, n]) + a[n] v1 + u[n] v2 + b~
  out     = relu(y)

where W~ are the BN-folded conv weights, v1 = W1~ s0, v2 = W2~ s1.  No [N,N]
matrix is ever materialized.

Sharding: 8 cores; core i handles batch b = i//2, half h = i%2 of the N=4096
node dim (2048 columns each).  Each core reads the full x[b] once in
transposed bf16 layout (feeds the s0/s1 reduction) and its own half in
natural bf16 layout (for the main matmuls).  n-chunks are rolled per-core so
chunks 0..15 are always the core's own half -> identical SPMD program.

v5 changes over the v4 baseline (32.2us -> target ~22us):
 - all-bf16 datapath: xh/wvb/xa/wAB/out in bf16.  Halves the dominant DMA
   traffic (input 1.11MB -> 0.82MB, output 512KB -> 256KB) and removes the
   fp32r 4x/row PE penalty (moving free dim 128 < 256 threshold).
 - w arrives pre-transposed from host as wcol [128, 32] fp32 (contiguous,
   128B rows): kills the PE transpose + PSUM hop that gated tanh ~0.9us.
 - inputs split across both HW queues with the critical tensors first:
   sync = wcol, xt[0:16], wvb, xh[:1024]; scalar = wrow, xt[16:32],
   xh[1024:] (xt lands ~10.4 vs ~13.9 in v4).
 - d2 = reciprocal_approx_fast (custom DVE, no table, ~5x faster than the
   table reciprocal); d = sqrt(d2) on scalar as before.
 - ONE v-matmul (s01 [64,66] stationary with col64=s0, col65=s1) instead of
   two, and v2 lands in wAB[65, 0:64] (the y1 half).  v4 had v2 in the q
   half, which wrongly scaled the u*v2 term by d2 (benign at ~4e-4 rel, but
   free accuracy here).
 - relus on the scalar engine; STT epilogue writes bf16.
"""

import numpy as np

import concourse.bacc as bacc
import concourse.bass as bass
import concourse.tile as tile
from concourse import mybir

FP = mybir.dt.float32
BF = mybir.dt.bfloat16
B, C, N = 4, 64, 4096
NH = N // 2          # columns per core
JH = NH // 128       # 16 chunks per core half
JF = N // 128        # 32 chunks full N
BN_EPS = 1e-5


def build_nc():
    nc = bacc.Bacc()
    AF = mybir.ActivationFunctionType
    OP = mybir.AluOpType
    AX = mybir.AxisListType

    # DRAM I/O (per-core shards supplied via in_maps)
    xt = nc.dram_tensor("xt", [128, JF, C], BF, kind="ExternalInput")
    # xh rows 0:64 = x half; row 66 = ones (host-baked); rows 64:65 zeros
    # (overwritten on device by the a/u row scatters)
    xh = nc.dram_tensor("xh", [67, NH], BF, kind="ExternalInput")
    wcol = nc.dram_tensor("wcol", [128, JF], FP, kind="ExternalInput")
    wrow = nc.dram_tensor("wrow", [JH, 128], FP, kind="ExternalInput")
    wvb = nc.dram_tensor("wvb", [67, 2 * C], BF, kind="ExternalInput")
    out = nc.dram_tensor("out", [128, JH, C], BF, kind="ExternalOutput")

    with tile.TileContext(nc) as tc:
        with (
            tc.tile_pool(name="sb", bufs=1) as sb,
            tc.tile_pool(name="ps", bufs=1, space="PSUM") as ps,
        ):
            # SBUF tiles
            xt_sb = sb.tile([128, JF, C], BF, name="xt_sb")
            # xa: rows 0:64 = x half (natural), 64 = a, 65 = u, 66 = 1
            xa = sb.tile([67, NH], BF, name="xa")
            wcol_sb = sb.tile([128, JF], FP, name="wcol_sb")
            wrow_sb = sb.tile([JH, 128], FP, name="wrow_sb")
            # wAB rows 0:64 = [W1~T | W2~T] (host), 64 = [v1|0], 65 = [v2|0]
            # (device; host preloads zeros), 66 = [b~ | 0] (host)
            wAB = sb.tile([67, 2 * C], BF, name="wAB")
            tcol = sb.tile([128, JF], FP, name="tcol")
            acol = sb.tile([128, JF], FP, name="acol")
            d2col = sb.tile([128, JF], FP, name="d2col")
            dcol = sb.tile([128, JF], FP, name="dcol")
            au_bf = sb.tile([128, 2 * JF], BF, name="au_bf")
            trow = sb.tile([JH, 128], FP, name="trow")
            arow_bf = sb.tile([JH, 128], BF, name="arow_bf")
            d2row = sb.tile([JH, 128], FP, name="d2row")
            drow = sb.tile([JH, 128], FP, name="drow")
            urow_bf = sb.tile([JH, 128], BF, name="urow_bf")
            ones_bf = sb.tile([128, 128], BF, name="ones_bf")
            apart = sb.tile([128, 1], FP, name="apart")
            apart_r = sb.tile([128, 2], BF, name="apart_r")
            sS = sb.tile([128, 1], FP, name="sS")
            # v-matmul stationaries: 66 cols of zeros except col 64 = s0 (A)
            # resp. col 65 = s1 (B) -> v1/v2 at PSUM partitions 64/65.  The
            # two v-matmuls accumulate into the same [66, 64] PSUM region so
            # row 64 = v1, row 65 = v2 both land in cols 0:64 (one aligned
            # 2-partition copy; engines can't start writes at partition 65).
            s0A = sb.tile([C, 66], BF, name="s0A")
            s1B = sb.tile([C, 66], BF, name="s1B")
            t1 = sb.tile([128, JH * C], FP, name="t1")
            yp = sb.tile([128, JH * C], BF, name="yp")
            yo = sb.tile([128, JH * C], BF, name="yo")

            # PSUM tiles (each padded to a bank; 7 banks total)
            p_sm = ps.tile([128, 2], FP, name="p_sm")
            p_s = ps.tile([C, 2], FP, name="p_s")
            p_v = ps.tile([66, C], FP, name="p_v")
            p_yq = [ps.tile([128, 512], FP, name=f"p_yq_{g}") for g in range(4)]

            # ---- DMA issues -------------------------------------------------
            # sync HW queue: wcol first (gates the whole scalar chain), then
            # its half of xt (gates the s-reduction), wvb, its half of xh.
            nc.sync.dma_start(wcol_sb[:], wcol[:])
            nc.sync.dma_start(xt_sb[:, 0:16, :], xt[:, 0:16, :])
            nc.sync.dma_start(wAB[:], wvb[:])
            nc.sync.dma_start(xa[:, 0:1024], xh[:, 0:1024])
            # scalar HW queue: wrow first (gates the row chain), then the
            # other xt half, the other xh half.  tanh runs between the xt and
            # xh descriptor writes so it isn't delayed past wcol's landing.
            nc.scalar.dma_start(wrow_sb[:], wrow[:])
            nc.scalar.dma_start(xt_sb[:, 16:32, :], xt[:, 16:32, :])

            # ---- tanh (scalar; wcol/wrow land first on their queues) -------
            with tc.high_priority():
                nc.scalar.activation(tcol[:], wcol_sb[:], AF.Tanh)
                nc.scalar.activation(trow[:], wrow_sb[:], AF.Tanh)
            nc.scalar.dma_start(xa[:, 1024:2048], xh[:, 1024:2048])

            # ---- constants (vector prologue) -------------------------------
            nc.vector.memset(ones_bf[:], 1.0)
            nc.vector.memset(s0A[:], 0.0)
            nc.vector.memset(s1B[:], 0.0)

            # ---- column chain: a, S, d2, d, u ------------------------------
            nc.vector.tensor_scalar_max(acol[:], tcol[:], 0.0)
            au_v = au_bf[:].rearrange("p (c s) -> p c s", s=2)
            nc.vector.tensor_copy(au_v[:, :, 0], acol[:])
            nc.vector.tensor_reduce(apart[:], acol[:], axis=AX.X, op=OP.add)
            nc.vector.tensor_copy(apart_r[:], apart[:].broadcast_to([128, 2]))
            # S broadcast to all partitions (tiny matmul; high priority so the
            # list scheduler doesn't sink it behind the s accumulation group)
            with tc.high_priority():
                nc.tensor.matmul(
                    p_sm[:], ones_bf[:], apart_r[:], start=True, stop=True
                )
            # sS copy only feeds the gpsimd row path; the column chain
            # reads S straight from PSUM
            nc.vector.tensor_copy(sS[:], p_sm[:, 0:1])
            # row-path relu here fills the vector gap while S broadcasts
            # (bf16 out so the scatter needs no cast and can use a HW queue
            # -- SWDGE completion semaphores cost ~1.3us before consumers)
            nc.vector.tensor_scalar_max(arow_bf[:], trow[:], 0.0)
            # t = a*S + 1 ; d2 = 1/t ; d = sqrt(d2) ; u = d*a
            nc.vector.tensor_scalar(
                tcol[:], acol[:], p_sm[:, 0:1], 1.0, op0=OP.mult, op1=OP.add
            )
            nc.vector.reciprocal_approx_fast(d2col[:], tcol[:])
            nc.scalar.sqrt(dcol[:], d2col[:])
            nc.vector.tensor_mul(au_v[:, :, 1], dcol[:], acol[:])

            # ---- row chain (a/u rows of xa).  t-row on gpsimd so the
            # in-order vector stream doesn't stall; sqrt on scalar; scatters
            # on SWDGE (HW-queue scatters wedged the scheduler into a ~10us
            # stall in the v5.6 trace -- don't move them back).
            nc.gpsimd.dma_start(xa[64:65, :], arow_bf[:])
            nc.gpsimd.tensor_scalar(
                trow[:], arow_bf[:], sS[0:JH, :], 1.0, op0=OP.mult, op1=OP.add
            )
            nc.vector.reciprocal_approx_fast(d2row[:], trow[:])
            nc.scalar.sqrt(drow[:], d2row[:])
            nc.vector.tensor_mul(urow_bf[:], drow[:], arow_bf[:])
            nc.gpsimd.dma_start(xa[65:66, :], urow_bf[:])

            # ---- s0/s1 reduction over full N (PE, accumulate in PSUM) ------
            for j in range(JF):
                nc.tensor.matmul(
                    p_s[:],
                    xt_sb[:, j, :],
                    au_bf[:, 2 * j : 2 * j + 2],
                    start=(j == 0),
                    stop=(j == JF - 1),
                )
            # v-matmuls accumulate into one region: row 64 = v1, row 65 = v2,
            # both in cols 0:64 (v2 against the W2~ block -> correct, unlike
            # v4 where v2 sat in the q half and got d2-scaled).  The copies
            # carry late ready-time hints so the scheduler doesn't slot them
            # early in the in-order vector stream.
            with tc.tile_wait_until(0.0115):
                nc.scalar.copy(s0A[:, 64:65], p_s[:, 0:1])
                nc.vector.tensor_copy(s1B[:, 65:66], p_s[:, 1:2])
            nc.tensor.matmul(
                p_v[:], s0A[:], wAB[0:C, 0:C], start=True, stop=False
            )
            nc.tensor.matmul(
                p_v[:], s1B[:], wAB[0:C, C : 2 * C], start=False, stop=True
            )
            with tc.tile_wait_until(0.0125):
                nc.vector.tensor_copy(wAB[64:66, 0:C], p_v[64:66, 0:C])

            # ---- main matmuls: one [67,128]x[67,128] mm per chunk.
            # out columns 0:64 = y1 (conv1 + rank-1 + bias), 64:128 = q (conv2)
            for j in range(JH):
                grp, jj = divmod(j, 4)
                nc.tensor.matmul(
                    p_yq[grp][:, 128 * jj : 128 * (jj + 1)],
                    xa[:, 128 * j : 128 * (j + 1)],
                    wAB[:],
                    start=True, stop=True,
                )

            # ---- epilogue: yo = relu(q * d2 + y1) --------------------------
            # Two wide vector ops per 4-chunk group instead of 16 per-chunk
            # STTs + 4 scalar y1-copies: t1 = q * d2 (d2 free-broadcast over
            # the 64 out channels), yp = t1 + y1 with y1 read straight from
            # PSUM (only one PSUM operand per op).
            for g in range(4):
                pv = p_yq[g][:].rearrange("p (j c) -> p j c", c=2 * C)
                t1v = t1[:, 256 * g : 256 * (g + 1)].rearrange(
                    "p (j c) -> p j c", c=C
                )
                ypv = yp[:, 256 * g : 256 * (g + 1)].rearrange(
                    "p (j c) -> p j c", c=C
                )
                d2v = (
                    d2col[:, 4 * g : 4 * (g + 1)]
                    .unsqueeze(2)
                    .broadcast_to([128, 4, C])
                )
                nc.vector.tensor_tensor(
                    t1v, pv[:, :, C : 2 * C], d2v, op=OP.mult
                )
                nc.vector.tensor_tensor(
                    ypv, t1v, pv[:, :, 0:C], op=OP.add
                )
            for g in range(4):
                nc.scalar.activation(
                    yo[:, 256 * g : 256 * (g + 1)],
                    yp[:, 256 * g : 256 * (g + 1)],
                    AF.Relu,
                )
                if g < 3:
                    eng = nc.scalar if g % 2 == 0 else nc.sync
                    eng.dma_start(
                        out[:, 4 * g : 4 * (g + 1), :],
                        yo[:, 256 * g : 256 * (g + 1)].rearrange(
                            "p (j c) -> p j c", c=C
                        ),
                    )
                else:
                    # the last group's transfer is exec-critical: split by
                    # partitions across both HW queues
                    nc.sync.dma_start(
                        out[0:64, 12:16, :],
                        yo[0:64, 768:1024].rearrange("p (j c) -> p j c", c=C),
                    )
                    nc.scalar.dma_start(
                        out[64:128, 12:16, :],
                        yo[64:128, 768:1024].rearrange("p (j c) -> p j c", c=C),
                    )
    nc.compile()
    return nc


def make_in_maps(x, w, conv_w, conv_b, bn_gamma, bn_beta, bn_mean, bn_var):
    import ml_dtypes

    BF_NP = ml_dtypes.bfloat16
    x = np.asarray(x, np.float32)
    w = np.asarray(w, np.float32)
    conv_w = np.asarray(conv_w, np.float32)
    conv_b = np.asarray(conv_b, np.float32)
    bn_gamma = np.asarray(bn_gamma, np.float32)
    bn_beta = np.asarray(bn_beta, np.float32)
    bn_mean = np.asarray(bn_mean, np.float32)
    bn_var = np.asarray(bn_var, np.float32)

    scale = bn_gamma / np.sqrt(bn_var + BN_EPS)
    wmat = conv_w * scale[:, None]                       # [64, 128] BN-folded
    w1t = np.ascontiguousarray(wmat[:, :C].T)            # [c, o]
    w2t = np.ascontiguousarray(wmat[:, C:].T)
    wvb = np.zeros((67, 2 * C), np.float32)
    wvb[0:C] = np.concatenate([w1t, w2t], axis=1)
    wvb[66, :C] = conv_b * scale + bn_beta - bn_mean * scale
    wvb = wvb.astype(BF_NP)

    in_maps = []
    for i in range(8):
        b, h = divmod(i, 2)
        xb = x[b, :, :, 0]                               # [64, 4096]
        order = np.roll(np.arange(JF), -JH * h)          # own half first
        xt_jpc = np.ascontiguousarray(xb.T).reshape(JF, 128, C)
        xt_pjc = np.ascontiguousarray(
            xt_jpc[order].transpose(1, 0, 2).astype(BF_NP)
        )
        xhb = np.zeros((67, NH), np.float32)
        xhb[0:C] = xb[:, NH * h : NH * (h + 1)]
        xhb[66] = 1.0
        xhb = np.ascontiguousarray(xhb.astype(BF_NP))
        wcol = np.ascontiguousarray(w[b].reshape(JF, 128)[order].T)
        wrow = np.ascontiguousarray(w[b][NH * h : NH * (h + 1)].reshape(JH, 128))
        in_maps.append(
            {
                "xt": xt_pjc,
                "xh": xhb,
                "wcol": wcol,
                "wrow": wrow,
                "wvb": wvb,
            }
        )
    return in_maps


def assemble_out(results):
    out = np.empty((B, C, N), np.float32)
    for i in range(8):
        b, h = divmod(i, 2)
        blk = np.asarray(results[i]["out"]).astype(np.float32)  # [128, 16, 64]
        y_half = blk.transpose(1, 0, 2).reshape(NH, C)   # row = 128*j + p
        out[b, :, NH * h : NH * (h + 1)] = y_half.T
    return out[..., None]


_NC = None


def kernel(**inputs):
    global _NC
    from concourse.bass_utils import run_bass_kernel_spmd

    if _NC is None:
        _NC = build_nc()
    in_maps = make_in_maps(**inputs)
    res = run_bass_kernel_spmd(_NC, in_maps, list(range(8)))
    return assemble_out(res.results)


# revision 24
# speedup vs baseline: 1.5368x; 1.5363x over previous
"""Trainium2 Bass kernel for a CGNS block (GNN message passing).

Math: the reference builds A = a a^T + I (rank-1 + identity), L = D^-1/2 A D^-1/2,
then out = relu(BN(conv1x1(cat[x@A, (L@x^T)^T]))).  Exploiting the rank-1
structure, with a = relu(tanh(w)), S = sum(a), d_n = 1/sqrt(a_n*S + 1),
u = d*a, s0 = x@a, s1 = x@u, the whole block collapses to

  y[:, n] = W1~ x[:, n] + d2[n] * (W2~ x[:, n]) + a[n] v1 + u[n] v2 + b~
  out     = relu(y)

where W~ are the BN-folded conv weights, v1 = W1~ s0, v2 = W2~ s1.  No [N,N]
matrix is ever materialized.

Sharding: 8 cores; core i handles batch b = i//2, half h = i%2 of the N=4096
node dim (2048 columns each).  Each core reads the full x[b] once in
transposed bf16 layout (feeds the s0/s1 reduction) and its own half in
natural bf16 layout (for the main matmuls).  n-chunks are rolled per-core so
chunks 0..15 are always the core's own half -> identical SPMD program.

v5 changes over the v4 baseline (32.2us -> target ~22us):
 - all-bf16 datapath: xh/wvb/xa/wAB/out in bf16.  Halves the dominant DMA
   traffic (input 1.11MB -> 0.82MB, output 512KB -> 256KB) and removes the
   fp32r 4x/row PE penalty (moving free dim 128 < 256 threshold).
 - w arrives pre-transposed from host as wcol [128, 32] fp32 (contiguous,
   128B rows): kills the PE transpose + PSUM hop that gated tanh ~0.9us.
 - inputs split across both HW queues with the critical tensors first:
   sync = wcol, xt[0:16], wvb, xh[:1024]; scalar = wrow, xt[16:32],
   xh[1024:] (xt lands ~10.4 vs ~13.9 in v4).
 - d2 = reciprocal_approx_fast (custom DVE, no table, ~5x faster than the
   table reciprocal); d = sqrt(d2) on scalar as before.
 - ONE v-matmul (s01 [64,66] stationary with col64=s0, col65=s1) instead of
   two, and v2 lands in wAB[65, 0:64] (the y1 half).  v4 had v2 in the q
   half, which wrongly scaled the u*v2 term by d2 (benign at ~4e-4 rel, but
   free accuracy here).
 - relus on the scalar engine; STT epilogue writes bf16.
"""

import numpy as np

import concourse.bacc as bacc
import concourse.bass as bass
import concourse.tile as tile
from concourse import mybir

FP = mybir.dt.float32
BF = mybir.dt.bfloat16
B, C, N = 4, 64, 4096
NH = N // 2          # columns per core
JH = NH // 128       # 16 chunks per core half
JF = N // 128        # 32 chunks full N
BN_EPS = 1e-5


def build_nc():
    nc = bacc.Bacc()
    AF = mybir.ActivationFunctionType
    OP = mybir.AluOpType
    AX = mybir.AxisListType

    # DRAM I/O (per-core shards supplied via in_maps)
    xt = nc.dram_tensor("xt", [128, JF, C], BF, kind="ExternalInput")
    # NOTE: keep DRAM->SBUF transfer partition dims divisible by 16!  A
    # 67-row transfer doesn't spray across the 16 DMA engines -- every
    # packet serializes on engine 0 at ~21GB/s (v5.7 trace: +12us).
    xh = nc.dram_tensor("xh", [C, NH], BF, kind="ExternalInput")
    wcol = nc.dram_tensor("wcol", [128, JF], FP, kind="ExternalInput")
    wrow = nc.dram_tensor("wrow", [JH, 128], FP, kind="ExternalInput")
    wvb = nc.dram_tensor("wvb", [67, 2 * C], BF, kind="ExternalInput")
    out = nc.dram_tensor("out", [128, JH, C], BF, kind="ExternalOutput")

    with tile.TileContext(nc) as tc:
        with (
            tc.tile_pool(name="sb", bufs=1) as sb,
            tc.tile_pool(name="ps", bufs=1, space="PSUM") as ps,
        ):
            # SBUF tiles
            xt_sb = sb.tile([128, JF, C], BF, name="xt_sb")
            # xa: rows 0:64 = x half (natural), 64 = a, 65 = u, 66 = 1
            xa = sb.tile([67, NH], BF, name="xa")
            wcol_sb = sb.tile([128, JF], FP, name="wcol_sb")
            wrow_sb = sb.tile([JH, 128], FP, name="wrow_sb")
            # wAB rows 0:64 = [W1~T | W2~T] (host), 64 = [v1|0], 65 = [v2|0]
            # (device; host preloads zeros), 66 = [b~ | 0] (host)
            wAB = sb.tile([67, 2 * C], BF, name="wAB")
            tcol = sb.tile([128, JF], FP, name="tcol")
            acol = sb.tile([128, JF], FP, name="acol")
            d2col = sb.tile([128, JF], FP, name="d2col")
            dcol = sb.tile([128, JF], FP, name="dcol")
            au_bf = sb.tile([128, 2 * JF], BF, name="au_bf")
            trow = sb.tile([JH, 128], FP, name="trow")
            obf = sb.tile([JH, 128], BF, name="obf")
            arow_bf = sb.tile([JH, 128], BF, name="arow_bf")
            d2row = sb.tile([JH, 128], FP, name="d2row")
            drow = sb.tile([JH, 128], FP, name="drow")
            urow_bf = sb.tile([JH, 128], BF, name="urow_bf")
            ones_bf = sb.tile([128, 128], BF, name="ones_bf")
            apart = sb.tile([128, 1], FP, name="apart")
            apart_r = sb.tile([128, 2], BF, name="apart_r")
            sS = sb.tile([128, 1], FP, name="sS")
            # v-matmul stationaries: 66 cols of zeros except col 64 = s0 (A)
            # resp. col 65 = s1 (B) -> v1/v2 at PSUM partitions 64/65.  The
            # two v-matmuls accumulate into the same [66, 64] PSUM region so
            # row 64 = v1, row 65 = v2 both land in cols 0:64 (one aligned
            # 2-partition copy; engines can't start writes at partition 65).
            s0A = sb.tile([C, 66], BF, name="s0A")
            s1B = sb.tile([C, 66], BF, name="s1B")
            t1 = sb.tile([128, JH * C], FP, name="t1")
            yp = sb.tile([128, JH * C], BF, name="yp")
            yo = sb.tile([128, JH * C], BF, name="yo")

            # PSUM tiles (each padded to a bank; 7 banks total)
            p_sm = ps.tile([128, 2], FP, name="p_sm")
            p_s = ps.tile([C, 2], FP, name="p_s")
            p_v = ps.tile([66, C], FP, name="p_v")
            p_yq = [ps.tile([128, 512], FP, name=f"p_yq_{g}") for g in range(4)]

            # ---- DMA issues -------------------------------------------------
            # sync HW queue: wcol first (gates the whole scalar chain), then
            # its half of xt (gates the s-reduction), wvb, its half of xh.
            nc.sync.dma_start(wcol_sb[:], wcol[:])
            nc.sync.dma_start(xt_sb[:, 0:16, :], xt[:, 0:16, :])
            # wvb split 64+3 so the big part sprays across DMA engines
            nc.sync.dma_start(wAB[0:64, :], wvb[0:64, :])
            nc.sync.dma_start(wAB[64:67, :], wvb[64:67, :])
            nc.sync.dma_start(xa[0:C, 0:1024], xh[:, 0:1024])
            # scalar HW queue: wrow first (gates the row chain), then the
            # other xt half, the other xh half.  tanh runs between the xt and
            # xh descriptor writes so it isn't delayed past wcol's landing.
            nc.scalar.dma_start(wrow_sb[:], wrow[:])
            nc.scalar.dma_start(xt_sb[:, 16:32, :], xt[:, 16:32, :])

            # ---- tanh (scalar; wcol/wrow land first on their queues) -------
            with tc.high_priority():
                nc.scalar.activation(tcol[:], wcol_sb[:], AF.Tanh)
                nc.scalar.activation(trow[:], wrow_sb[:], AF.Tanh)
            nc.scalar.dma_start(xa[0:C, 1024:2048], xh[:, 1024:2048])

            # ---- constants (vector prologue) + ones row of xa --------------
            nc.vector.memset(ones_bf[:], 1.0)
            nc.vector.memset(obf[:], 1.0)
            nc.vector.memset(s0A[:], 0.0)
            nc.vector.memset(s1B[:], 0.0)
            nc.gpsimd.dma_start(
                xa[66:67, :].rearrange("r (j p) -> r j p", p=128), obf[:]
            )

            # ---- column chain: a, S, d2, d, u ------------------------------
            nc.vector.tensor_scalar_max(acol[:], tcol[:], 0.0)
            au_v = au_bf[:].rearrange("p (c s) -> p c s", s=2)
            nc.vector.tensor_copy(au_v[:, :, 0], acol[:])
            nc.vector.tensor_reduce(apart[:], acol[:], axis=AX.X, op=OP.add)
            nc.vector.tensor_copy(apart_r[:], apart[:].broadcast_to([128, 2]))
            # S broadcast to all partitions (tiny matmul; high priority so the
            # list scheduler doesn't sink it behind the s accumulation group)
            with tc.high_priority():
                nc.tensor.matmul(
                    p_sm[:], ones_bf[:], apart_r[:], start=True, stop=True
                )
            # sS copy only feeds the gpsimd row path; the column chain
            # reads S straight from PSUM
            nc.vector.tensor_copy(sS[:], p_sm[:, 0:1])
            # row-path relu here fills the vector gap while S broadcasts
            # (bf16 out so the scatter needs no cast and can use a HW queue
            # -- SWDGE completion semaphores cost ~1.3us before consumers)
            nc.vector.tensor_scalar_max(arow_bf[:], trow[:], 0.0)
            # t = a*S + 1 ; d2 = 1/t ; d = sqrt(d2) ; u = d*a
            nc.vector.tensor_scalar(
                tcol[:], acol[:], p_sm[:, 0:1], 1.0, op0=OP.mult, op1=OP.add
            )
            nc.vector.reciprocal_approx_fast(d2col[:], tcol[:])
            nc.scalar.sqrt(dcol[:], d2col[:])
            nc.vector.tensor_mul(au_v[:, :, 1], dcol[:], acol[:])

            # ---- row chain (a/u rows of xa).  t-row on gpsimd so the
            # in-order vector stream doesn't stall; sqrt on scalar; scatters
            # on SWDGE (HW-queue scatters wedged the scheduler into a ~10us
            # stall in the v5.6 trace -- don't move them back).
            nc.gpsimd.dma_start(xa[64:65, :], arow_bf[:])
            nc.gpsimd.tensor_scalar(
                trow[:], arow_bf[:], sS[0:JH, :], 1.0, op0=OP.mult, op1=OP.add
            )
            nc.vector.reciprocal_approx_fast(d2row[:], trow[:])
            nc.scalar.sqrt(drow[:], d2row[:])
            nc.vector.tensor_mul(urow_bf[:], drow[:], arow_bf[:])
            nc.gpsimd.dma_start(xa[65:66, :], urow_bf[:])

            # ---- s0/s1 reduction over full N (PE, accumulate in PSUM) ------
            for j in range(JF):
                nc.tensor.matmul(
                    p_s[:],
                    xt_sb[:, j, :],
                    au_bf[:, 2 * j : 2 * j + 2],
                    start=(j == 0),
                    stop=(j == JF - 1),
                )
            # v-matmuls accumulate into one region: row 64 = v1, row 65 = v2,
            # both in cols 0:64 (v2 against the W2~ block -> correct, unlike
            # v4 where v2 sat in the q half and got d2-scaled).  The copies
            # carry late ready-time hints so the scheduler doesn't slot them
            # early in the in-order vector stream.
            with tc.tile_wait_until(0.0115):
                nc.scalar.copy(s0A[:, 64:65], p_s[:, 0:1])
                nc.vector.tensor_copy(s1B[:, 65:66], p_s[:, 1:2])
            nc.tensor.matmul(
                p_v[:], s0A[:], wAB[0:C, 0:C], start=True, stop=False
            )
            nc.tensor.matmul(
                p_v[:], s1B[:], wAB[0:C, C : 2 * C], start=False, stop=True
            )
            with tc.tile_wait_until(0.0125):
                nc.vector.tensor_copy(wAB[64:66, 0:C], p_v[64:66, 0:C])

            # ---- main matmuls: one [67,128]x[67,128] mm per chunk.
            # out columns 0:64 = y1 (conv1 + rank-1 + bias), 64:128 = q (conv2)
            for j in range(JH):
                grp, jj = divmod(j, 4)
                nc.tensor.matmul(
                    p_yq[grp][:, 128 * jj : 128 * (jj + 1)],
                    xa[:, 128 * j : 128 * (j + 1)],
                    wAB[:],
                    start=True, stop=True,
                )

            # ---- epilogue: yo = relu(q * d2 + y1) --------------------------
            # Two wide vector ops per 4-chunk group instead of 16 per-chunk
            # STTs + 4 scalar y1-copies: t1 = q * d2 (d2 free-broadcast over
            # the 64 out channels), yp = t1 + y1 with y1 read straight from
            # PSUM (only one PSUM operand per op).
            for g in range(4):
                pv = p_yq[g][:].rearrange("p (j c) -> p j c", c=2 * C)
                t1v = t1[:, 256 * g : 256 * (g + 1)].rearrange(
                    "p (j c) -> p j c", c=C
                )
                ypv = yp[:, 256 * g : 256 * (g + 1)].rearrange(
                    "p (j c) -> p j c", c=C
                )
                d2v = (
                    d2col[:, 4 * g : 4 * (g + 1)]
                    .unsqueeze(2)
                    .broadcast_to([128, 4, C])
                )
                nc.vector.tensor_tensor(
                    t1v, pv[:, :, C : 2 * C], d2v, op=OP.mult
                )
                nc.vector.tensor_tensor(
                    ypv, t1v, pv[:, :, 0:C], op=OP.add
                )
            for g in range(4):
                nc.scalar.activation(
                    yo[:, 256 * g : 256 * (g + 1)],
                    yp[:, 256 * g : 256 * (g + 1)],
                    AF.Relu,
                )
                if g < 3:
                    eng = nc.scalar if g % 2 == 0 else nc.sync
                    eng.dma_start(
                        out[:, 4 * g : 4 * (g + 1), :],
                        yo[:, 256 * g : 256 * (g + 1)].rearrange(
                            "p (j c) -> p j c", c=C
                        ),
                    )
                else:
                    # the last group's transfer is exec-critical: split by
                    # partitions across both HW queues
                    nc.sync.dma_start(
                        out[0:64, 12:16, :],
                        yo[0:64, 768:1024].rearrange("p (j c) -> p j c", c=C),
                    )
                    nc.scalar.dma_start(
                        out[64:128, 12:16, :],
                        yo[64:128, 768:1024].rearrange("p (j c) -> p j c", c=C),
                    )
    nc.compile()
    return nc


def make_in_maps(x, w, conv_w, conv_b, bn_gamma, bn_beta, bn_mean, bn_var):
    import ml_dtypes

    BF_NP = ml_dtypes.bfloat16
    x = np.asarray(x, np.float32)
    w = np.asarray(w, np.float32)
    conv_w = np.asarray(conv_w, np.float32)
    conv_b = np.asarray(conv_b, np.float32)
    bn_gamma = np.asarray(bn_gamma, np.float32)
    bn_beta = np.asarray(bn_beta, np.float32)
    bn_mean = np.asarray(bn_mean, np.float32)
    bn_var = np.asarray(bn_var, np.float32)

    scale = bn_gamma / np.sqrt(bn_var + BN_EPS)
    wmat = conv_w * scale[:, None]                       # [64, 128] BN-folded
    w1t = np.ascontiguousarray(wmat[:, :C].T)            # [c, o]
    w2t = np.ascontiguousarray(wmat[:, C:].T)
    wvb = np.zeros((67, 2 * C), np.float32)
    wvb[0:C] = np.concatenate([w1t, w2t], axis=1)
    wvb[66, :C] = conv_b * scale + bn_beta - bn_mean * scale
    wvb = wvb.astype(BF_NP)

    in_maps = []
    for i in range(8):
        b, h = divmod(i, 2)
        xb = x[b, :, :, 0]                               # [64, 4096]
        order = np.roll(np.arange(JF), -JH * h)          # own half first
        xt_jpc = np.ascontiguousarray(xb.T).reshape(JF, 128, C)
        xt_pjc = np.ascontiguousarray(
            xt_jpc[order].transpose(1, 0, 2).astype(BF_NP)
        )
        xhb = np.ascontiguousarray(
            xb[:, NH * h : NH * (h + 1)].astype(BF_NP)
        )
        wcol = np.ascontiguousarray(w[b].reshape(JF, 128)[order].T)
        wrow = np.ascontiguousarray(w[b][NH * h : NH * (h + 1)].reshape(JH, 128))
        in_maps.append(
            {
                "xt": xt_pjc,
                "xh": xhb,
                "wcol": wcol,
                "wrow": wrow,
                "wvb": wvb,
            }
        )
    return in_maps


def assemble_out(results):
    out = np.empty((B, C, N), np.float32)
    for i in range(8):
        b, h = divmod(i, 2)
        blk = np.asarray(results[i]["out"]).astype(np.float32)  # [128, 16, 64]
        y_half = blk.transpose(1, 0, 2).reshape(NH, C)   # row = 128*j + p
        out[b, :, NH * h : NH * (h + 1)] = y_half.T
    return out[..., None]


_NC = None


def kernel(**inputs):
    global _NC
    from concourse.bass_utils import run_bass_kernel_spmd

    if _NC is None:
        _NC = build_nc()
    in_maps = make_in_maps(**inputs)
    res = run_bass_kernel_spmd(_NC, in_maps, list(range(8)))
    return assemble_out(res.results)
